# revision 2
# baseline (speedup 1.0000x reference)
"""MoE FFN (top-1 switch routing) on 8 Trainium2 NeuronCores.

Strategy: expert parallelism, one expert per core (E == n_cores == 8).
The host computes the router argmax (dispatch decision only), gathers each
expert's tokens (padded to a fixed capacity), and each core runs the full
expert FFN -- including the router softmax that produces the top-1
probability scale -- on its own tokens. The host scatters per-core outputs
back to token order.

All matmuls run in float32r (TF32-like, full PE rate for moving dim >= 256).
"""
import sys
import numpy as np

sys.path.insert(0, "/root/.axon_site")

import concourse.bass as bass
import concourse.bacc as bacc
import concourse.mybir as mybir
import concourse.tile as tile
import concourse.bass_utils as bass_utils

P = 128          # partitions
D = 1024         # d_model
MLP = 4096       # mlp dim
E = 8            # experts == cores
B, T = 4, 1024
N_TOK = B * T
C = 640          # per-expert token capacity (seed-0 max count is 608)
KD = D // P      # 8 k-tiles over D
KM = MLP // P    # 32 k-tiles over MLP
TT = C // P      # 5 token tiles
TC = 320         # FFN1 moving-dim token chunk (>=256 keeps f32r at full rate)
NCH = C // TC    # 2 chunks
MB = 512         # W1 streaming block (mlp cols)
DH = 512         # FFN2 output column half
F32 = mybir.dt.float32
F32R = mybir.dt.float32r
AX = mybir.AxisListType.X
AF = mybir.ActivationFunctionType

_cached = {}


def build_nc():
    nc = bacc.Bacc("TRN2", target_bir_lowering=False, debug=False)

    xgT_d = nc.declare_dram_parameter("xgT", [D, C], F32R, isOutput=False)
    w1_d = nc.declare_dram_parameter("w1", [D, MLP], F32R, isOutput=False)
    w2_d = nc.declare_dram_parameter("w2", [MLP, D], F32R, isOutput=False)
    wg_d = nc.declare_dram_parameter("wg", [D, E], F32R, isOutput=False)
    b1_d = nc.declare_dram_parameter("b1pm", [P, KM], F32, isOutput=False)
    bgr_d = nc.declare_dram_parameter("bgr", [P, E], F32, isOutput=False)
    b2r_d = nc.declare_dram_parameter("b2r", [P, D], F32, isOutput=False)
    y_d = nc.declare_dram_parameter("y", [C, D], F32, isOutput=True)

    xgT_r = xgT_d[:].rearrange("(ko p) t -> p ko t", p=P)   # (128, KD, C)
    w1_r = w1_d[:].rearrange("(ko p) m -> p ko m", p=P)     # (128, KD, MLP)
    w2_r = w2_d[:].rearrange("(ko p) d -> p ko d", p=P)     # (128, KM, D)
    wg_r = wg_d[:].rearrange("(ko p) e -> p ko e", p=P)     # (128, KD, E)

    with tile.TileContext(nc) as tc:
        with (
            tc.tile_pool(name="const", bufs=1) as cpool,
            tc.tile_pool(name="hpool", bufs=1) as hpool,
            tc.tile_pool(name="w1p", bufs=2) as w1p,
            tc.tile_pool(name="w2p", bufs=4) as w2p,
            tc.tile_pool(name="tmp", bufs=4) as tmp,
            tc.tile_pool(name="yout", bufs=4) as ypool,
        ):
            xgT = cpool.tile([P, KD, C], F32R, tag="xgT")
            nc.sync.dma_start(out=xgT[:], in_=xgT_r)
            wg = cpool.tile([P, KD, E], F32R, tag="wg")
            nc.sync.dma_start(out=wg[:], in_=wg_r)
            b1 = cpool.tile([P, KM], F32, tag="b1")
            nc.sync.dma_start(out=b1[:], in_=b1_d[:])
            bgr = cpool.tile([P, E], F32, tag="bgr")
            nc.sync.dma_start(out=bgr[:], in_=bgr_d[:])
            b2r = cpool.tile([P, D], F32, tag="b2r")
            nc.sync.dma_start(out=b2r[:], in_=b2r_d[:])
            hT = hpool.tile([P, KM, C], F32R, tag="hT")
            p_scale = cpool.tile([P, TT], F32, tag="p_scale")

            # ---- Router: p = max(softmax(xg @ wg + bg)) = 1/sum(exp(l - max)) ----
            with tc.tile_pool(name="ps_lg", bufs=2, space="PSUM") as ps_lg:
                for t in range(TT):
                    lg = ps_lg.tile([P, E], F32, tag="lg")
                    for k in range(KD):
                        nc.tensor.matmul(
                            lg[:],
                            xgT[:, k, t * P:(t + 1) * P],
                            wg[:, k, :],
                            start=(k == 0),
                            stop=(k == KD - 1),
                        )
                    lg_sb = tmp.tile([P, E], F32, tag="lg_sb")
                    nc.vector.tensor_add(lg_sb[:], lg[:], bgr[:])
                    negm = tmp.tile([P, 1], F32, tag="negm")
                    nc.vector.reduce_max(negm[:], lg_sb[:], axis=AX, negate=True)
                    et = tmp.tile([P, E], F32, tag="et")
                    nc.scalar.activation(et[:], lg_sb[:], AF.Exp, bias=negm[:])
                    s = tmp.tile([P, 1], F32, tag="s")
                    nc.vector.reduce_sum(s[:], et[:], axis=AX)
                    nc.vector.reciprocal(p_scale[:, t:t + 1], s[:])

            # ---- FFN1: hT = relu(W1^T x^T + b1), mlp on partitions ----
            with tc.tile_pool(name="ps_h", bufs=3, space="PSUM") as ps_h:
                for mb in range(MLP // MB):
                    w1t = w1p.tile([P, KD, MB], F32R, tag="w1t")
                    nc.sync.dma_start(out=w1t[:], in_=w1_r[:, :, mb * MB:(mb + 1) * MB])
                    for ml in range(MB // P):
                        m = mb * (MB // P) + ml
                        for c in range(NCH):
                            hp = ps_h.tile([P, TC], F32, tag="hp")
                            for k in range(KD):
                                nc.tensor.matmul(
                                    hp[:],
                                    w1t[:, k, ml * P:(ml + 1) * P],
                                    xgT[:, k, c * TC:(c + 1) * TC],
                                    start=(k == 0),
                                    stop=(k == KD - 1),
                                )
                            nc.scalar.activation(
                                hT[:, m, c * TC:(c + 1) * TC], hp[:],
                                AF.Relu, bias=b1[:, m:m + 1],
                            )

            # ---- FFN2: y = (h @ W2 + b2) * p, tokens on partitions ----
            with tc.tile_pool(name="ps_y", bufs=TT, space="PSUM") as ps_y:
                for dh in range(D // DH):
                    yps = [
                        ps_y.tile([P, DH], F32, tag="yps", name=f"yps{dh}_{t}")
                        for t in range(TT)
                    ]
                    for k4 in range(KM // 4):
                        w2t = w2p.tile([P, 4, DH], F32R, tag="w2t")
                        nc.sync.dma_start(
                            out=w2t[:],
                            in_=w2_r[:, k4 * 4:(k4 + 1) * 4, dh * DH:(dh + 1) * DH],
                        )
                        for kk in range(4):
                            k = k4 * 4 + kk
                            for t in range(TT):
                                nc.tensor.matmul(
                                    yps[t][:],
                                    hT[:, k, t * P:(t + 1) * P],
                                    w2t[:, kk, :],
                                    start=(k == 0),
                                    stop=(k == KM - 1),
                                )
                    for t in range(TT):
                        ysb = ypool.tile([P, DH], F32, tag="ysb")
                        nc.vector.tensor_add(ysb[:], yps[t][:], b2r[:, dh * DH:(dh + 1) * DH])
                        yfin = ypool.tile([P, DH], F32, tag="yfin")
                        nc.scalar.activation(
                            yfin[:], ysb[:], AF.Copy, scale=p_scale[:, t:t + 1]
                        )
                        nc.sync.dma_start(
                            out=y_d[t * P:(t + 1) * P, dh * DH:(dh + 1) * DH],
                            in_=yfin[:],
                        )
    nc.compile()
    return nc


def _route_host(x_flat, w_gate, b_gate):
    logits = x_flat @ w_gate + b_gate
    return logits.argmax(-1), logits


def _ffn_host(xs, w_gate, b_gate, W1e, b1e, W2e, b2e):
    """Numpy fallback for capacity-overflow tokens (rarely used)."""
    logits = xs @ w_gate + b_gate
    m = logits.max(-1, keepdims=True)
    e = np.exp(logits - m)
    p = (e.max(-1) / e.sum(-1)).astype(np.float32)
    h = np.maximum(xs @ W1e + b1e, 0.0)
    return ((h @ W2e + b2e) * p[:, None]).astype(np.float32)


def kernel(x, w_gate, b_gate, W1, b1, W2, b2):
    x = np.ascontiguousarray(x, np.float32)
    w_gate = np.ascontiguousarray(w_gate, np.float32)
    b_gate = np.ascontiguousarray(b_gate, np.float32)
    W1 = np.ascontiguousarray(W1, np.float32)
    b1 = np.ascontiguousarray(b1, np.float32)
    W2 = np.ascontiguousarray(W2, np.float32)
    b2 = np.ascontiguousarray(b2, np.float32)

    x_flat = x.reshape(N_TOK, D)
    idx, _ = _route_host(x_flat, w_gate, b_gate)

    bgr = np.ascontiguousarray(np.broadcast_to(b_gate, (P, E)), np.float32)

    ids = []
    in_maps = []
    for e in range(E):
        ids_e = np.nonzero(idx == e)[0]
        ids.append(ids_e)
        cnt = min(len(ids_e), C)
        xg = np.zeros((C, D), np.float32)
        xg[:cnt] = x_flat[ids_e[:cnt]]
        xgT = np.ascontiguousarray(xg.T)
        b1pm = np.ascontiguousarray(b1[e].reshape(KM, P).T)
        b2r = np.ascontiguousarray(np.broadcast_to(b2[e], (P, D)), np.float32)
        in_maps.append({
            "xgT": xgT, "w1": W1[e], "w2": W2[e], "wg": w_gate,
            "b1pm": b1pm, "bgr": bgr, "b2r": b2r,
        })

    if "nc" not in _cached:
        _cached["nc"] = build_nc()
    nc = _cached["nc"]

    res = bass_utils.run_bass_kernel_spmd(nc, in_maps, list(range(E)))

    out_flat = np.empty((N_TOK, D), np.float32)
    for e in range(E):
        ids_e = ids[e]
        cnt = min(len(ids_e), C)
        out_flat[ids_e[:cnt]] = res.results[e]["y"][:cnt]
        if len(ids_e) > cnt:  # capacity overflow: host fallback
            rest = ids_e[cnt:]
            out_flat[rest] = _ffn_host(
                x_flat[rest], w_gate, b_gate, W1[e], b1[e], W2[e], b2[e]
            )
    return out_flat.reshape(B, T, D)


# revision 3
# speedup vs baseline: 1.0764x; 1.0764x over previous
"""MoE FFN (top-1 switch routing) on 8 Trainium2 NeuronCores.

Strategy: expert parallelism, one expert per core (E == n_cores == 8).
The host computes the router argmax (dispatch decision only), gathers each
expert's tokens (padded to a fixed capacity C), and each core runs the full
expert FFN -- including the router softmax that produces the top-1
probability scale -- on its own tokens. The host scatters per-core outputs
back to token order.

Matmuls run in bf16 (full PE rate + fast weight load); set MM_DTYPE to
float32r for a TF32-like higher-precision variant.
"""
import sys
import numpy as np
import ml_dtypes

sys.path.insert(0, "/root/.axon_site")

import concourse.bass as bass
import concourse.bacc as bacc
import concourse.mybir as mybir
import concourse.tile as tile
import concourse.bass_utils as bass_utils

P = 128          # partitions
D = 1024         # d_model
MLP = 4096       # mlp dim
E = 8            # experts == cores
B, T = 4, 1024
N_TOK = B * T
C = 640          # per-expert token capacity (seed-0 max count is 608)
KD = D // P      # 8 k-tiles over D
KM = MLP // P    # 32 k-tiles over MLP
TT = C // P      # 5 token tiles
TC = 320         # FFN1 moving-dim token chunk (>=256 keeps f32r at full rate)
NCH = C // TC    # 2 chunks
MB = 512         # W1 streaming block (mlp cols)
DH = 512         # FFN2 output column half
F32 = mybir.dt.float32
AX = mybir.AxisListType.X
AF = mybir.ActivationFunctionType

MM_DTYPE = mybir.dt.bfloat16      # or mybir.dt.float32r
_NP_MM = ml_dtypes.bfloat16 if MM_DTYPE == mybir.dt.bfloat16 else np.float32

_cached = {}


def build_nc():
    nc = bacc.Bacc("TRN2", target_bir_lowering=False, debug=False)
    MMD = MM_DTYPE

    xgT_d = nc.declare_dram_parameter("xgT", [D, C], MMD, isOutput=False)
    w1_d = nc.declare_dram_parameter("w1", [D, MLP], MMD, isOutput=False)
    w2_d = nc.declare_dram_parameter("w2", [MLP, D], MMD, isOutput=False)
    wg_d = nc.declare_dram_parameter("wg", [D, E], MMD, isOutput=False)
    b1_d = nc.declare_dram_parameter("b1pm", [P, KM], F32, isOutput=False)
    bgr_d = nc.declare_dram_parameter("bgr", [P, E], F32, isOutput=False)
    b2r_d = nc.declare_dram_parameter("b2r", [P, D], F32, isOutput=False)
    y_d = nc.declare_dram_parameter("y", [C, D], F32, isOutput=True)

    xgT_r = xgT_d[:].rearrange("(ko p) t -> p ko t", p=P)   # (128, KD, C)
    w1_r = w1_d[:].rearrange("(ko p) m -> p ko m", p=P)     # (128, KD, MLP)
    w2_r = w2_d[:].rearrange("(ko p) d -> p ko d", p=P)     # (128, KM, D)
    wg_r = wg_d[:].rearrange("(ko p) e -> p ko e", p=P)     # (128, KD, E)

    with tile.TileContext(nc) as tc:
        with (
            tc.tile_pool(name="const", bufs=1) as cpool,
            tc.tile_pool(name="hpool", bufs=1) as hpool,
            tc.tile_pool(name="w1p", bufs=2) as w1p,
            tc.tile_pool(name="w2p", bufs=4) as w2p,
            tc.tile_pool(name="tmp", bufs=4) as tmp,
            tc.tile_pool(name="yout", bufs=4) as ypool,
        ):
            # activation-queue DMAs (parallel with weight streams on sync)
            xgT = cpool.tile([P, KD, C], MMD, tag="xgT")
            for kh in range(2):
                nc.scalar.dma_start(
                    out=xgT[:, kh * (KD // 2):(kh + 1) * (KD // 2), :],
                    in_=xgT_r[:, kh * (KD // 2):(kh + 1) * (KD // 2), :],
                )
            wg = cpool.tile([P, KD, E], MMD, tag="wg")
            nc.scalar.dma_start(out=wg[:], in_=wg_r)
            b1 = cpool.tile([P, KM], F32, tag="b1")
            nc.scalar.dma_start(out=b1[:], in_=b1_d[:])
            bgr = cpool.tile([P, E], F32, tag="bgr")
            nc.scalar.dma_start(out=bgr[:], in_=bgr_d[:])
            b2r = cpool.tile([P, D], F32, tag="b2r")
            nc.scalar.dma_start(out=b2r[:], in_=b2r_d[:])
            hT = hpool.tile([P, KM, C], MMD, tag="hT")
            p_scale = cpool.tile([P, TT], F32, tag="p_scale")

            # ---- Router: p = max(softmax(xg @ wg + bg)) = 1/sum(exp(l - max)) ----
            with tc.tile_pool(name="ps_lg", bufs=2, space="PSUM") as ps_lg:
                for t in range(TT):
                    lg = ps_lg.tile([P, E], F32, tag="lg")
                    for k in range(KD):
                        nc.tensor.matmul(
                            lg[:],
                            xgT[:, k, t * P:(t + 1) * P],
                            wg[:, k, :],
                            start=(k == 0),
                            stop=(k == KD - 1),
                        )
                    lg_sb = tmp.tile([P, E], F32, tag="lg_sb")
                    nc.vector.tensor_add(lg_sb[:], lg[:], bgr[:])
                    negm = tmp.tile([P, 1], F32, tag="negm")
                    nc.vector.reduce_max(negm[:], lg_sb[:], axis=AX, negate=True)
                    et = tmp.tile([P, E], F32, tag="et")
                    nc.scalar.activation(et[:], lg_sb[:], AF.Exp, bias=negm[:])
                    s = tmp.tile([P, 1], F32, tag="s")
                    nc.vector.reduce_sum(s[:], et[:], axis=AX)
                    nc.vector.reciprocal(p_scale[:, t:t + 1], s[:])

            # ---- FFN1: hT = relu(W1^T x^T + b1), mlp on partitions ----
            with tc.tile_pool(name="ps_h", bufs=3, space="PSUM") as ps_h:
                for mb in range(MLP // MB):
                    w1t = w1p.tile([P, KD, MB], MMD, tag="w1t")
                    nc.sync.dma_start(out=w1t[:], in_=w1_r[:, :, mb * MB:(mb + 1) * MB])
                    for ml in range(MB // P):
                        m = mb * (MB // P) + ml
                        for c in range(NCH):
                            hp = ps_h.tile([P, TC], F32, tag="hp")
                            for k in range(KD):
                                nc.tensor.matmul(
                                    hp[:],
                                    w1t[:, k, ml * P:(ml + 1) * P],
                                    xgT[:, k, c * TC:(c + 1) * TC],
                                    start=(k == 0),
                                    stop=(k == KD - 1),
                                )
                            nc.scalar.activation(
                                hT[:, m, c * TC:(c + 1) * TC], hp[:],
                                AF.Relu, bias=b1[:, m:m + 1],
                            )

            # ---- FFN2: y = (h @ W2 + b2) * p, tokens on partitions ----
            with tc.tile_pool(name="ps_y", bufs=TT, space="PSUM") as ps_y:
                for dh in range(D // DH):
                    yps = [
                        ps_y.tile([P, DH], F32, tag="yps", name=f"yps{dh}_{t}")
                        for t in range(TT)
                    ]
                    for k4 in range(KM // 4):
                        w2t = w2p.tile([P, 4, DH], MMD, tag="w2t")
                        nc.sync.dma_start(
                            out=w2t[:],
                            in_=w2_r[:, k4 * 4:(k4 + 1) * 4, dh * DH:(dh + 1) * DH],
                        )
                        for kk in range(4):
                            k = k4 * 4 + kk
                            for t in range(TT):
                                nc.tensor.matmul(
                                    yps[t][:],
                                    hT[:, k, t * P:(t + 1) * P],
                                    w2t[:, kk, :],
                                    start=(k == 0),
                                    stop=(k == KM - 1),
                                )
                    for t in range(TT):
                        ysb = ypool.tile([P, DH], F32, tag="ysb")
                        nc.vector.tensor_add(ysb[:], yps[t][:], b2r[:, dh * DH:(dh + 1) * DH])
                        yfin = ypool.tile([P, DH], F32, tag="yfin")
                        nc.scalar.activation(
                            yfin[:], ysb[:], AF.Copy, scale=p_scale[:, t:t + 1]
                        )
                        nc.sync.dma_start(
                            out=y_d[t * P:(t + 1) * P, dh * DH:(dh + 1) * DH],
                            in_=yfin[:],
                        )
    nc.compile()
    return nc


def _ffn_host(xs, w_gate, b_gate, W1e, b1e, W2e, b2e):
    """Numpy fallback for capacity-overflow tokens (rarely used)."""
    logits = xs @ w_gate + b_gate
    m = logits.max(-1, keepdims=True)
    e = np.exp(logits - m)
    p = (e.max(-1) / e.sum(-1)).astype(np.float32)
    h = np.maximum(xs @ W1e + b1e, 0.0)
    return ((h @ W2e + b2e) * p[:, None]).astype(np.float32)


def kernel(x, w_gate, b_gate, W1, b1, W2, b2):
    x = np.ascontiguousarray(x, np.float32)
    w_gate = np.ascontiguousarray(w_gate, np.float32)
    b_gate = np.ascontiguousarray(b_gate, np.float32)
    W1 = np.ascontiguousarray(W1, np.float32)
    b1 = np.ascontiguousarray(b1, np.float32)
    W2 = np.ascontiguousarray(W2, np.float32)
    b2 = np.ascontiguousarray(b2, np.float32)

    x_flat = x.reshape(N_TOK, D)
    idx = (x_flat @ w_gate + b_gate).argmax(-1)

    bgr = np.ascontiguousarray(np.broadcast_to(b_gate, (P, E)), np.float32)
    wg_mm = w_gate.astype(_NP_MM)

    ids = []
    in_maps = []
    for e in range(E):
        ids_e = np.nonzero(idx == e)[0]
        ids.append(ids_e)
        cnt = min(len(ids_e), C)
        xg = np.zeros((C, D), np.float32)
        xg[:cnt] = x_flat[ids_e[:cnt]]
        xgT = np.ascontiguousarray(xg.T).astype(_NP_MM)
        b1pm = np.ascontiguousarray(b1[e].reshape(KM, P).T)
        b2r = np.ascontiguousarray(np.broadcast_to(b2[e], (P, D)), np.float32)
        in_maps.append({
            "xgT": xgT, "w1": W1[e].astype(_NP_MM), "w2": W2[e].astype(_NP_MM),
            "wg": wg_mm, "b1pm": b1pm, "bgr": bgr, "b2r": b2r,
        })

    if "nc" not in _cached:
        _cached["nc"] = build_nc()
    nc = _cached["nc"]

    res = bass_utils.run_bass_kernel_spmd(nc, in_maps, list(range(E)))

    out_flat = np.empty((N_TOK, D), np.float32)
    for e in range(E):
        ids_e = ids[e]
        cnt = min(len(ids_e), C)
        out_flat[ids_e[:cnt]] = res.results[e]["y"][:cnt]
        if len(ids_e) > cnt:  # capacity overflow: host fallback
            rest = ids_e[cnt:]
            out_flat[rest] = _ffn_host(
                x_flat[rest], w_gate, b_gate, W1[e], b1[e], W2[e], b2[e]
            )
    return out_flat.reshape(B, T, D)
